# revision 1
# baseline (speedup 1.0000x reference)
"""Trainium2 Bass kernel for DifferentialAttention (B=2, S=2048, DIM=2048).

Sharding: 8 cores = 2 batches x 4 head-groups (4 heads each). Per core:
  - QKV projection (column-parallel slices of wq/wk/wv) + RoPE on device
  - differential attention for its 4 heads
  - row-parallel wo partial product; host sums the 4 partials per batch.

v2 design (cost-model driven):
  * scores in [q, k] layout: psc[128 q, 1024 k] per (h, branch, qtile).
    exp on Act engine with accum_out -> per-query denominators for FREE
    (no M=1 denominator matmuls on PE: saves ~55us PE per core).
  * et [q, k] transposed to [k, q] chunks via DMA-transpose (XBAR, 14ns
    per 16x128 tile, off-engine) for the pv matmuls.
  * pv out [q 128, dv 128]: per-query quantities (d1, d2, rr, rs) are
    per-PARTITION columns -> stage C uses tensor_scalar per-partition
    scalar APs; no broadcast matmuls, no row DMAs.
  * u' = lam*(d1/d2)*pv2 - pv1 = -u; the sign is folded into a host-side
    negation of wo. rsqrt via Quake bit-trick + 2 Newton steps on DVE
    (keeps Act on {Copy, Exp} = one act table set, no table reloads).
  * wo needs attn^T: small DMA-transpose per qtile, then 4x4 matmuls.
  * RoPE: head-dim perm puts rope partners 16 rows apart inside each
    32-partition quadrant, so the cross-partner swap is a single DVE
    stream_shuffle (no DMA, no extra matmul); the cos/sin multiplies run
    on the otherwise-idle GPSIMD engine.
  * B/C software-pipelined with skew (scores qt | pv qt-2 | wo qt-3),
    interleaved at (h, j)-beat granularity so the in-order PE queue
    never head-blocks on Act/DMA results; the s-half-1 V and Q
    projections are woven into the first three steps' beats (their psum
    borrows the banks the wo stage does not need until step 3).
  * out DMAs, woven weight loads and wot go through the GPSIMD (SWDGE)
    queue so the SP sequencer carries only the XBAR transposes.

Per-core layouts (partition dim first):
  QT [128,4,S]: q heads; rows = [branch j | quadrant q | E(16) O(16)],
     row j*64+q*32+c*16+i <-> head-dim 2*(16q+i)+c.
  KT [128,2,S]: same for the 2 kv heads.
  Vn [128,16,256]: v natural [s, dv] layout, s-tile major.
  et [128 q, 8 hj, 1024 k]; etT [128 k, 64, 128 q] via DMA transpose.
"""

import math
import numpy as np
import ml_dtypes
import concourse.bass as bass
import concourse.tile as tile
from concourse import bacc, mybir
from concourse.bass_utils import run_bass_kernel_spmd
from contextlib import ExitStack

F32 = mybir.dt.float32
BF16 = mybir.dt.bfloat16
AF = mybir.ActivationFunctionType
ALU = mybir.AluOpType

DIM = 2048
S = 2048
B = 2
HD = 64          # rope head dim
EPS = 1e-5
SCALE = HD ** -0.5
NCORES = 8
NQT = S // 128   # 16 query tiles

TRACE = False    # set by test.py to collect an NTFF profile
LAST_RESULTS = None

# stream_shuffle mask: swap 16-row halves within each 32-partition quadrant
_SWAP16 = list(range(16, 32)) + list(range(0, 16))


# ---------------------------------------------------------------- device program

def build_program(lam: float):
    nc = bacc.Bacc("TRN2", target_bir_lowering=False, debug=False,
                   num_devices=NCORES)
    io = {}
    for name, shape, d in [
        ("xT", [DIM, S], BF16), ("wq_s", [DIM, 512], BF16),
        ("wk_s", [DIM, 256], BF16), ("wv_s", [DIM, 256], BF16),
        ("wo_s", [512, DIM], BF16),
        ("cs128", [128, S], BF16), ("sn128", [128, S], BF16),
    ]:
        io[name] = nc.dram_tensor(name, shape, d, kind="ExternalInput").ap()
    out = nc.dram_tensor("out", [S, DIM], BF16, kind="ExternalOutput").ap()

    with tile.TileContext(nc) as tc:
        _body(tc, io, out, lam)
    nc.compile()
    return nc


def _body(tc, io, out, lam):
    nc = tc.nc
    with ExitStack() as top:
        persist = top.enter_context(tc.tile_pool(name="persist", bufs=1))
        QT = persist.tile([128, 4, S], BF16)
        KT = persist.tile([128, 2, S], BF16)
        Vn = persist.tile([128, 16, 256], BF16)

        # B pools first (bottom of the pool stacks) so the A pools can
        # be released mid-B in LIFO order
        bctx = ExitStack()
        b = _stage_bc_pools(tc, bctx)
        actx = ExitStack()
        apsum = ExitStack()
        a = _stage_a_setup(tc, actx, apsum, io)
        # A emission: kv+q of half 0, kv of half 1. q(half 1) is woven
        # into the first B steps (B only needs K/V complete + Q half 0).
        _emit_xh(tc, a, 0)
        _emit_k(tc, a, 0, KT)
        _emit_v(tc, a, 0, Vn)
        _emit_xh(tc, a, 1)
        _emit_q(tc, a, 0, QT, KT, Vn, act_evac=True)
        a["xh"] = a["xh_next"]
        _emit_k(tc, a, 1, KT)
        # free A's x buffer and wide psum pools; the woven q(1)
        # projection streams x chunks and gets a single-buffer psum pool
        apsum.close()
        b["pss"] = bctx.enter_context(tc.tile_pool(name="pss", bufs=2,
                                                   space="PSUM"))
        b["ppvp"] = bctx.enter_context(tc.tile_pool(name="ppv", bufs=1,
                                                    space="PSUM"))
        a["ps"] = actx.enter_context(tc.tile_pool(name="pq1", bufs=1,
                                                  space="PSUM"))
        with bctx:
            _stage_bc(tc, io, out, QT, KT, Vn, lam, a, actx, b)


def _stage_a_setup(tc, ctx, psctx, io):
    nc = tc.nc
    a = {}
    a["wp"] = ctx.enter_context(tc.tile_pool(name="wqk", bufs=2))
    a["trig"] = ctx.enter_context(tc.tile_pool(name="trig", bufs=1))
    a["tmp"] = ctx.enter_context(tc.tile_pool(name="ropetmp", bufs=1))
    a["xp"] = ctx.enter_context(tc.tile_pool(name="xh", bufs=2))
    a["wvp"] = ctx.enter_context(tc.tile_pool(name="wvp", bufs=1))
    a["ps"] = psctx.enter_context(tc.tile_pool(name="ps_qk", bufs=2,
                                               space="PSUM"))
    a["psv"] = psctx.enter_context(tc.tile_pool(name="ps_v", bufs=2,
                                                space="PSUM"))
    a["xT3"] = io["xT"].rearrange("(a p) s -> p a s", p=128)
    a["wq3"] = io["wq_s"].rearrange("(a p) c -> p a c", p=128)
    a["wk3"] = io["wk_s"].rearrange("(a p) c -> p a c", p=128)

    # first x half chunked so the first matmuls can start early
    xh0 = a["xp"].tile([128, 16, 1024], BF16, tag="xh", name="xh0")
    nc.sync.dma_start(xh0[:, 0:2, :], a["xT3"][:, 0:2, 0:1024])
    wk00 = a["wp"].tile([128, 16, 128], BF16, tag="w", name="wk00")
    nc.sync.dma_start(wk00[:, 0:4, :], a["wk3"][:, 0:4, 0:128])
    nc.sync.dma_start(xh0[:, 2:4, :], a["xT3"][:, 2:4, 0:1024])
    nc.sync.dma_start(wk00[:, 4:16, :], a["wk3"][:, 4:16, 0:128])
    wv_all = a["wvp"].tile([128, 16, 256], BF16)
    wv3 = io["wv_s"].rearrange("(a p) c -> p a c", p=128)
    nc.sync.dma_start(wv_all[:, 0:8, :], wv3[:, 0:8, :])
    # first trig halves lead the bulk x chunks: the s-half-0 ropes need
    # them ~6us before the second x chunks are consumed
    cs = a["trig"].tile([128, S], BF16)
    sn = a["trig"].tile([128, S], BF16)
    nc.sync.dma_start(cs[:, 0:1024], io["cs128"][:, 0:1024])
    nc.sync.dma_start(sn[:, 0:1024], io["sn128"][:, 0:1024])
    for c in range(1, 4):
        nc.sync.dma_start(xh0[:, c * 4:(c + 1) * 4, :],
                          a["xT3"][:, c * 4:(c + 1) * 4, 0:1024])
    a["xh0"] = xh0
    a["wk00"] = wk00

    nc.sync.dma_start(cs[:, 1024:2048], io["cs128"][:, 1024:2048])
    nc.sync.dma_start(sn[:, 1024:2048], io["sn128"][:, 1024:2048])
    a["cs"], a["sn"] = cs, sn
    nc.sync.dma_start(wv_all[:, 8:16, :], wv3[:, 8:16, :])
    a["wv_all"] = wv_all
    return a


def _emit_xh(tc, a, sq):
    if sq == 0:
        a["xh"] = a["xh0"]
        return
    nc = tc.nc
    xh = a["xp"].tile([128, 16, 1024], BF16, tag="xh", name=f"xh{sq}")
    nc.sync.dma_start(xh[:], a["xT3"][:, :, sq * 1024:(sq + 1) * 1024])
    a["xh_next"] = xh


def _rope(tc, a, pq, dest_ap, ssl, act_evac):
    """c0 = evac(pq); dest = c0*cs + shuffle(c0*sn)."""
    nc = tc.nc
    W = 1024
    c0 = a["tmp"].tile([128, W], BF16, tag="c0", name="c0")
    if act_evac:
        nc.scalar.activation(c0[:], pq[:], AF.Copy, bias=0.0, scale=1.0)
    else:
        # during the B overlap the Act engine is saturated with exps
        nc.vector.tensor_copy(c0[:], pq[:])
    t2 = a["tmp"].tile([128, W], BF16, tag="t2", name="t2")
    t2s = a["tmp"].tile([128, W], BF16, tag="t2s", name="t2s")
    nc.gpsimd.tensor_mul(t2[:], c0[:], a["sn"][:, ssl])
    nc.vector.stream_shuffle(t2s[:], t2[:], _SWAP16)
    nc.gpsimd.tensor_mul(c0[:], c0[:], a["cs"][:, ssl])
    nc.vector.tensor_add(dest_ap, c0[:], t2s[:])


def _emit_k(tc, a, sq, KT):
    """K projection (2 kv tiles) + rope for s-half sq."""
    nc = tc.nc
    W = 1024
    ssl = slice(sq * W, sq * W + W)
    xh = a["xh"]
    for kt_i in range(2):
        if sq == 0 and kt_i == 0:
            wct = a["wk00"]
        else:
            wct = a["wp"].tile([128, 16, 128], BF16, tag="w",
                               name=f"wk{sq}{kt_i}")
            nc.sync.dma_start(wct[:],
                              a["wk3"][:, :, kt_i * 128:(kt_i + 1) * 128])
        pq = a["ps"].tile([128, W], F32, tag="psqk", name="pqk")
        for dt_i in range(16):
            for nch in range(2):
                nsl = slice(nch * 512, (nch + 1) * 512)
                nc.tensor.matmul(pq[:, nsl], lhsT=wct[:, dt_i, :],
                                 rhs=xh[:, dt_i, nsl],
                                 start=(dt_i == 0), stop=(dt_i == 15))
        _rope(tc, a, pq, KT[:, kt_i, ssl], ssl, act_evac=True)


def _v_st_tasks(tc, a, sq, st, Vn, inline):
    """Closures for one V s-tile. In the woven (sq=1) path the psum is a
    256-column sub-slice of the shared single-buffer projection tile."""
    nc = tc.nc
    xh = a["xh"]
    state = {}

    def mms(lo, hi):
        def run():
            if lo == 0:
                if inline:
                    pv = a["psv"].tile([128, 256], F32, tag="psv",
                                       name="psv")
                else:
                    pvt = a["ps"].tile([128, 1024], F32, tag="psqk",
                                       name="psv1")
                    pv = pvt[:, 0:256]
                state["pv"] = pv
            for dt_i in range(lo, hi):
                nc.tensor.matmul(state["pv"],
                                 lhsT=xh[:, dt_i, st * 128:(st + 1) * 128],
                                 rhs=a["wv_all"][:, dt_i, :],
                                 start=(dt_i == 0), stop=(dt_i == 15))
        return run

    def copy():
        nc.vector.tensor_copy(Vn[:, sq * 8 + st, :], state["pv"])

    return [mms(0, 8), mms(8, 16), copy]


def _emit_v(tc, a, sq, Vn):
    for st in range(8):
        for task in _v_st_tasks(tc, a, sq, st, Vn, inline=True):
            task()


def _emit_q(tc, a, sq, QT, KT, Vn, act_evac):
    for ct in range(4):
        for task in _q_ct_tasks(tc, a, sq, ct, QT, act_evac):
            task()


def _q_ct_tasks(tc, a, sq, ct, QT, act_evac):
    """Closures for one Q head-tile projection: 16 matmul steps + rope.

    When a["xh"] is None (the B-overlap weave), each matmul streams its
    own x chunk from DRAM through a small rotating pool."""
    nc = tc.nc
    W = 1024
    ssl = slice(sq * W, sq * W + W)
    state = {}

    def load():
        wct = a["wp"].tile([128, 16, 128], BF16, tag="w",
                           name=f"wq{sq}{ct}")
        dma = nc.sync.dma_start if sq == 0 else nc.gpsimd.dma_start
        dma(wct[:], a["wq3"][:, :, ct * 128:(ct + 1) * 128])
        pq = a["ps"].tile([128, W], F32, tag="psqk", name="pqq")
        state["wct"], state["pq"] = wct, pq

    def mm(dt_i):
        def run():
            xsrc = a["xh"][:, dt_i, :]
            for nch in range(2):
                nsl = slice(nch * 512, (nch + 1) * 512)
                nc.tensor.matmul(state["pq"][:, nsl],
                                 lhsT=state["wct"][:, dt_i, :],
                                 rhs=xsrc[:, nsl],
                                 start=(dt_i == 0), stop=(dt_i == 15))
        return run

    def rope():
        _rope(tc, a, state["pq"], QT[:, ct, ssl], ssl, act_evac)

    return [load] + [mm(i) for i in range(16)] + [rope]


def _stage_bc_pools(tc, ctx):
    nc = tc.nc
    b = {"ctx": ctx}
    b["etpA"] = ctx.enter_context(tc.tile_pool(name="etpA", bufs=1))
    b["etpB"] = ctx.enter_context(tc.tile_pool(name="etpB", bufs=1))
    b["etTp"] = ctx.enter_context(tc.tile_pool(name="etTp", bufs=2))
    b["dcp"] = ctx.enter_context(tc.tile_pool(name="dcp", bufs=4))
    b["colp"] = ctx.enter_context(tc.tile_pool(name="colp", bufs=3))
    b["up"] = ctx.enter_context(tc.tile_pool(name="up", bufs=2))
    b["atp"] = ctx.enter_context(tc.tile_pool(name="atp", bufs=2))
    b["atTp"] = ctx.enter_context(tc.tile_pool(name="atTp", bufs=3))
    magp = ctx.enter_context(tc.tile_pool(name="magic", bufs=1))
    b["magicT"] = magp.tile([128, 4], mybir.dt.uint32, name="magicT")
    nc.gpsimd.memset(b["magicT"][:], 0x5F3759DF)
    return b


def _stage_bc(tc, io, out, QT, KT, Vn, lam, a, actx, b):
    """Attention + norm + wo, software-pipelined over 16 query tiles.

    step s: scores+exp+transpose(qt=s) | pv+norm(qt=s-2) | wo+out(qt=s-3).
    The three PE workloads interleave at (h, j)-beat granularity so the
    in-order PE queue always has ready work while Act runs the exps.
    The Q projection of s-half 1 is woven into steps 0-2 (its PSUM use
    borrows the banks the wo stage does not need until step 3).
    """
    nc = tc.nc
    # v+q of s-half 1, interleaved into the early-step beats; each q
    # weight load is hoisted one head-tile ahead of its matmuls
    proj = []
    for st in range(8):
        proj.extend(_v_st_tasks(tc, a, 1, st, Vn, inline=False))
    ct_tasks = [_q_ct_tasks(tc, a, 1, ct, QT, act_evac=False)
                for ct in range(4)]
    proj += [ct_tasks[0][0], ct_tasks[1][0]] + ct_tasks[0][1:]
    for ct in range(1, 4):
        if ct + 1 < 4:
            proj.append(ct_tasks[ct + 1][0])
        proj.extend(ct_tasks[ct][1:])
    proj_i = [0]

    def drain_proj(n):
        k = 0
        while k < n and proj_i[0] < len(proj):
            proj[proj_i[0]]()
            proj_i[0] += 1
            k += 1

    ctx = b["ctx"]
    etTp, dcp, colp = b["etTp"], b["dcp"], b["colp"]
    up, atp, atTp = b["up"], b["atp"], b["atTp"]
    pss, ppvp = b["pss"], b["ppvp"]
    magicT = b["magicT"]
    outwp = None
    psop = None
    if True:

        etT_t = [None] * NQT
        dc_t = [None] * NQT
        atT_t = [None] * NQT

        for step in range(NQT + 2):
            fr = step if step < NQT else None
            mid = step - 2 if 2 <= step < NQT + 2 else None
            bk = step - 3 if 3 <= step else None

            if step == 3:
                # q(half 1) projection is done; its A pools (and PSUM
                # banks) make room for the wo accumulators
                assert proj_i[0] == len(proj)
                actx.close()
                psop = ctx.enter_context(tc.tile_pool(name="pso", bufs=2,
                                                      space="PSUM"))
                outwp = ctx.enter_context(tc.tile_pool(name="outwp", bufs=2))

                wotp = ctx.enter_context(tc.tile_pool(name="wotp", bufs=1))
                wot = wotp.tile([128, 4, S], BF16, name="wot")
                nc.gpsimd.dma_start(wot[:], io["wo_s"].rearrange(
                    "(a p) c -> p a c", p=128))

            if fr is not None:
                etpool = [b["etpA"], b["etpB"]][fr % 2]
                et = etpool.tile([128, 8, 1024], BF16, tag="et", name="et")
                etT = etTp.tile([128, 64, 128], BF16, tag="etT")
                etT_t[fr] = etT
                dc = dcp.tile([128, 8], F32, tag="dc")
                dc_t[fr] = dc
            if mid is not None:
                dcm = dc_t[mid]
                rec = colp.tile([128, 4], F32, tag="rec")
                rrl = colp.tile([128, 4], F32, tag="rrl")
                msum = colp.tile([128, 4], F32, tag="msum")
                tcol = colp.tile([128, 4], F32, tag="tcol")
                ya = colp.tile([128, 4], F32, tag="ya")
                aa = colp.tile([128, 4], F32, tag="aa")
                shu = colp.tile([128, 4], mybir.dt.uint32, tag="shu")
                nc.vector.reciprocal(rec[:], dcm[:, 4:8])
                nc.vector.scalar_tensor_tensor(
                    rrl[:], dcm[:, 0:4], float(lam), rec[:],
                    op0=ALU.mult, op1=ALU.mult)
                # two psum tiles (h0+h1 / h2+h3, slots [j0, j1] per h);
                # pv beats alternate tiles (h order 0,2,1,3) so a pv
                # write never WAR-waits on the previous head's u' reads
                ppvA = ppvp.tile([128, 4, 128], F32, tag="ppvA")
                ppvB = ppvp.tile([128, 4, 128], F32, tag="ppvB")
                def ppv_ap(h, j):
                    t = ppvA if h < 2 else ppvB
                    return t[:, (h % 2) * 2 + j, :]
                u = up.tile([128, 4, 128], BF16, tag="u")
                usq = up.tile([128, 128], BF16, tag="usq")
                v2 = up.tile([128, 4, 128], BF16, tag="v2")
                at = atp.tile([128, 4, 128], BF16, tag="at")
            if bk is not None:
                atTb = atT_t[bk]
                outw = outwp.tile([128, 2048], BF16, tag="outw")

            H_ORDER = (0, 2, 1, 3)
            for beat in range(8):
                if step < 3:
                    drain_proj(5)
                # frontend: one (h, j) scores pair + exp(+denominator)
                if fr is not None:
                    hj = beat
                    j, h = divmod(hj, 4)
                    kvl, rho = h // 2, h % 2
                    jsl = slice(j * 64, j * 64 + 64)
                    qsl = slice(fr * 128, fr * 128 + 128)
                    psc = pss.tile([128, 1024], F32, tag="sc")
                    for nch in range(2):
                        nsl = slice(nch * 512, (nch + 1) * 512)
                        nc.tensor.matmul(
                            psc[:, nsl],
                            lhsT=QT[jsl, h, qsl],
                            rhs=KT[jsl, kvl,
                                   rho * 1024 + nch * 512:
                                   rho * 1024 + nch * 512 + 512],
                            start=True, stop=True)
                    nc.scalar.activation(et[:, hj, :], psc[:], AF.Exp,
                                         bias=0.0, scale=float(SCALE),
                                         accum_out=dc[:, hj:hj + 1])
                # middle: pv accumulation for one (h, j)
                if mid is not None:
                    h = H_ORDER[beat // 2]
                    j = beat % 2
                    hj = j * 4 + h
                    kvl, rho = h // 2, h % 2
                    etTm = etT_t[mid]
                    pdst = ppv_ap(h, j)
                    for kt in range(8):
                        nc.tensor.matmul(
                            pdst,
                            lhsT=etTm[:, hj * 8 + kt, :],
                            rhs=Vn[:, rho * 8 + kt,
                                   kvl * 128:(kvl + 1) * 128],
                            start=(kt == 0), stop=(kt == 7))
                    # u' for head h once both branches are accumulated
                    if j == 1:
                        nc.vector.tensor_scalar_mul(
                            v2[:, h, :], ppv_ap(h, 1), rrl[:, h:h + 1])
                        nc.vector.tensor_sub(u[:, h, :], v2[:, h, :],
                                             ppv_ap(h, 0))
                        nc.vector.scalar_tensor_tensor(
                            usq[:], u[:, h, :], 1.0, u[:, h, :],
                            op0=ALU.mult, op1=ALU.mult,
                            accum_out=msum[:, h:h + 1])
                # backend: one e-chunk of the wo matmul
                if bk is not None and beat < 4:
                    ech = beat
                    pso = psop.tile([128, 512], F32, tag="pso")
                    for r in range(4):
                        nc.tensor.matmul(
                            pso[:], lhsT=atTb[:, r, :],
                            rhs=wot[:, r, ech * 512:(ech + 1) * 512],
                            start=(r == 0), stop=(r == 3))
                    nc.vector.tensor_copy(outw[:, ech * 512:(ech + 1) * 512],
                                          pso[:])

            if step < 3:
                drain_proj(len(proj))
            if mid is not None:
                # rs = rsqrt(eps*d1^2 + sum(u^2)/128), Quake bit-trick +
                # two Newton steps on the DVE (keeps Act on {Copy, Exp}
                # only -> a single activation-table set, no reloads)
                nc.vector.scalar_tensor_tensor(
                    tcol[:], dcm[:, 0:4], float(EPS), dcm[:, 0:4],
                    op0=ALU.mult, op1=ALU.mult)
                nc.vector.scalar_tensor_tensor(
                    tcol[:], msum[:], float(1.0 / 128.0), tcol[:],
                    op0=ALU.mult, op1=ALU.add)
                nc.vector.tensor_scalar(shu[:], tcol[:].bitcast(
                    mybir.dt.uint32), 1, None, op0=ALU.arith_shift_right)
                nc.vector.tensor_sub(ya[:].bitcast(mybir.dt.uint32),
                                     magicT[:], shu[:])
                for _ in range(2):
                    nc.vector.tensor_mul(aa[:], ya[:], ya[:])
                    nc.vector.tensor_mul(aa[:], aa[:], tcol[:])
                    nc.vector.tensor_scalar(aa[:], aa[:], -0.5, 1.5,
                                            op0=ALU.mult, op1=ALU.add)
                    nc.vector.tensor_mul(ya[:], ya[:], aa[:])
                for h in range(4):
                    nc.vector.tensor_scalar_mul(at[:, h, :], u[:, h, :],
                                                ya[:, h:h + 1])
                atT = atTp.tile([128, 4, 128], BF16, tag="atT")
                atT_t[mid] = atT
                nc.sync.dma_start_transpose(atT[:], at[:])

            if fr is not None:
                nc.sync.dma_start_transpose(etT[:], et[:])

            if bk is not None:
                # Pool-issued (SWDGE) so the out write never queues behind
                # the transposes on the SP sequencer
                nc.gpsimd.dma_start(out[bk * 128:(bk + 1) * 128, :], outw[:])

        # compressed drain: the last qtile's wo follows immediately instead
        # of occupying its own pipeline step
        bk = NQT - 1
        atTb = atT_t[bk]
        outw = outwp.tile([128, 2048], BF16, tag="outw")
        for ech in range(4):
            pso = psop.tile([128, 512], F32, tag="pso")
            for r in range(4):
                nc.tensor.matmul(
                    pso[:], lhsT=atTb[:, r, :],
                    rhs=wot[:, r, ech * 512:(ech + 1) * 512],
                    start=(r == 0), stop=(r == 3))
            nc.vector.tensor_copy(outw[:, ech * 512:(ech + 1) * 512],
                                  pso[:])
        nc.gpsimd.dma_start(out[bk * 128:(bk + 1) * 128, :], outw[:])


# ---------------------------------------------------------------- host side

# row (q*32 + c*16 + i) within a branch <-> head-dim 2*(16q+i)+c
_PERM64 = np.empty(64, np.int64)
for _q in range(2):
    for _c in range(2):
        for _i in range(16):
            _PERM64[_q * 32 + _c * 16 + _i] = 2 * (16 * _q + _i) + _c


def make_core_inputs(core, x, wq, wk, wv, wo, subln_w, lambda_init,
                     freqs_cos, freqs_sin):
    b, g = divmod(core, 4)
    npdt = ml_dtypes.bfloat16
    qcols = np.empty(512, np.int64)
    for hl in range(4):
        for j in range(2):
            qcols[hl * 128 + j * 64:hl * 128 + j * 64 + 64] = \
                ((4 * g + hl) * 2 + j) * 64 + _PERM64
    kcols = np.empty(256, np.int64)
    for kvl in range(2):
        for j in range(2):
            kcols[kvl * 128 + j * 64:kvl * 128 + j * 64 + 64] = \
                ((2 * g + kvl) * 2 + j) * 64 + _PERM64
    vcols = np.arange(256) + 2 * g * 128

    cosT = np.ascontiguousarray(freqs_cos.T.astype(np.float32))  # [32, S]
    sinT = np.ascontiguousarray(freqs_sin.T.astype(np.float32))
    cs64 = np.concatenate([cosT[0:16], cosT[0:16],
                           cosT[16:32], cosT[16:32]], axis=0)
    sn64 = np.concatenate([sinT[0:16], -sinT[0:16],
                           sinT[16:32], -sinT[16:32]], axis=0)
    # wo rows carry subln*(1-lambda_init) and the global sign flip (u' = -u)
    wo_s = wo[512 * g: 512 * g + 512, :].astype(np.float32).copy()
    wo_s *= -np.tile(subln_w.astype(np.float32)
                     * (1.0 - np.float32(np.asarray(lambda_init)[0])),
                     4)[:, None]
    return {
        "xT": np.ascontiguousarray(x[b].T.astype(np.float32)).astype(npdt),
        "wq_s": np.ascontiguousarray(wq[:, qcols].astype(np.float32)).astype(npdt),
        "wk_s": np.ascontiguousarray(wk[:, kcols].astype(np.float32)).astype(npdt),
        "wv_s": np.ascontiguousarray(wv[:, vcols].astype(np.float32)).astype(npdt),
        "wo_s": wo_s.astype(npdt),
        "cs128": np.tile(cs64, (2, 1)).astype(npdt),
        "sn128": np.tile(sn64, (2, 1)).astype(npdt),
    }


def compute_lambda(lambda_q1, lambda_k1, lambda_q2, lambda_k2, lambda_init):
    l1 = np.exp(np.sum(np.float32(lambda_q1) * np.float32(lambda_k1),
                       dtype=np.float32))
    l2 = np.exp(np.sum(np.float32(lambda_q2) * np.float32(lambda_k2),
                       dtype=np.float32))
    return float(l1 - l2 + np.float32(np.asarray(lambda_init)[0]))


def kernel(x, wq, wk, wv, wo, lambda_q1, lambda_k1, lambda_q2, lambda_k2,
           lambda_init, subln_w, freqs_cos, freqs_sin):
    global LAST_RESULTS
    x = np.asarray(x); wq = np.asarray(wq); wk = np.asarray(wk)
    wv = np.asarray(wv); wo = np.asarray(wo)
    lam = compute_lambda(lambda_q1, lambda_k1, lambda_q2, lambda_k2, lambda_init)

    nc = build_program(lam)
    in_maps = [make_core_inputs(c, x, wq, wk, wv, wo,
                                np.asarray(subln_w), np.asarray(lambda_init),
                                np.asarray(freqs_cos), np.asarray(freqs_sin))
               for c in range(NCORES)]
    res = run_bass_kernel_spmd(nc, in_maps, list(range(NCORES)), trace=TRACE)
    LAST_RESULTS = res
    outs = [res.results[c]["out"] for c in range(NCORES)]
    full = np.empty((B, S, DIM), np.float32)
    for b in range(B):
        full[b] = (outs[4 * b].astype(np.float32)
                   + outs[4 * b + 1].astype(np.float32)
                   + outs[4 * b + 2].astype(np.float32)
                   + outs[4 * b + 3].astype(np.float32))
    return full



# revision 10
# speedup vs baseline: 1.0907x; 1.0907x over previous
"""Trainium2 Bass kernel for DifferentialAttention (B=2, S=2048, DIM=2048).

Sharding: 8 cores = 2 batches x 4 head-groups (4 heads each). Per core:
  - QKV projection + RoPE on device, differential attention for 4 heads,
  - row-parallel wo partial product; host sums the 4 partials per batch.

v3 design (cost-model driven):
  * Projections (Q/K/V) and wo run in fp8e4 DoubleRow perf mode (0.5
    cycles/row, 2 k-tiles per call -> 4x bf16 matmul throughput). Accuracy
    is held at bf16 level with a hi/lo split: x ~ (e4m3(16x)+e4m3(res))/16
    against w_hi, plus an x/2 copy against the 32x-boosted w residual.
    All three terms accumulate in one f32 psum group (scale 64).
  * Scores are produced TRANSPOSED ([k,q]: lhsT=KT tile, rhs=QT tile,
    8 k-tile matmuls into a [128,8,128] psum tile) so the exp on the Act
    engine writes etT directly in the layout PV needs -- the former
    [q,k]->[k,q] XBAR DMA transposes (120us of serialized DMA) vanish.
  * Softmax denominators come for free from PV: V carries an appended
    ones column (rhs width 129), so ppv[:,128] = sum_k p. No accum_out
    on the exps -> cheaper Act instructions.
  * wo: atT is split post-transpose into (e4m3(8at), residual, /32 copy)
    and multiplied against wo_hi/wo_lo fp8 in DoubleRow (6 calls/nch).
  * u' = lam*(d1/d2)*pv2 - pv1 fused into one scalar_tensor_tensor with
    a per-partition scalar; rsqrt via Quake bit-trick + 2 Newton steps.
  * The 64x fp8 scale cancels algebraically: exp scale = SCALE/4096,
    rsqrt arg scaled so ya = rsqrt(true)/8 (t8 = 8*at for the fp8 split),
    host divides the summed partials by 128.
  * No A/BC weave: the A phase is PE-dense fp8; BC steps are balanced
    PE ~9.4us / Act ~8.8us / DVE ~8.4us. Pipeline skew 1 (scores/exp at
    step s, PV+norm at s-1, wo+out at s-2).

Per-core layouts (partition dim first):
  QT [128,4,S]: q heads; rows = [branch j | quadrant q | E(16) O(16)],
     row j*64+q*32+c*16+i <-> head-dim 2*(16q+i)+c. Values 64x scaled.
  KT [128,2,S]: same for the 2 kv heads.
  Vn [128,16,2,129]: v natural [s, dv] layout + ones column, s-tile major.
  etT [128,8,8,128]: [k-in-tile, hj, ktile, q] per query tile.
"""

import math
import numpy as np
import ml_dtypes
import concourse.bass as bass
import concourse.tile as tile
from concourse import bacc, mybir
from concourse.bass_utils import run_bass_kernel_spmd
from contextlib import ExitStack

F32 = mybir.dt.float32
BF16 = mybir.dt.bfloat16
FP8 = mybir.dt.float8e4
AF = mybir.ActivationFunctionType
ALU = mybir.AluOpType
DR = mybir.MatmulPerfMode.DoubleRow

DIM = 2048
S = 2048
B = 2
HD = 64          # rope head dim
EPS = 1e-5
SCALE = HD ** -0.5
ESC = SCALE / 4096.0   # exp scale on the 64x-scaled scores psum
NCORES = 8
NQT = S // 128   # 16 query tiles

TRACE = False
LAST_RESULTS = None
DBG = None       # set to a dict to get QT/KT/Vn/etT/u/t8 debug outputs

# stream_shuffle mask: swap 16-row halves within each 32-partition quadrant
_SWAP16 = list(range(16, 32)) + list(range(0, 16))

F8NP = ml_dtypes.float8_e4m3
BFNP = ml_dtypes.bfloat16


# ---------------------------------------------------------------- device program

def build_program(lam: float):
    nc = bacc.Bacc("TRN2", target_bir_lowering=False, debug=False,
                   num_devices=NCORES)
    io = {}
    for name, shape, d in [
        ("xhi", [DIM, S], FP8), ("xlo", [DIM, S], FP8), ("xh2", [DIM, S], FP8),
        ("wq_hi", [DIM, 512], FP8), ("wq_lo", [DIM, 512], FP8),
        ("wk_hi", [DIM, 256], FP8), ("wk_lo", [DIM, 256], FP8),
        ("wv_hi", [DIM, 256], FP8), ("wv_lo", [DIM, 256], FP8),
        ("wo_hi", [512, DIM], FP8), ("wo_lo", [512, DIM], FP8),
        ("cs128", [128, S], BF16), ("sn128", [128, S], BF16),
    ]:
        io[name] = nc.dram_tensor(name, shape, d, kind="ExternalInput").ap()
    out = nc.dram_tensor("out", [S, DIM], BF16, kind="ExternalOutput").ap()

    if DBG is not None:
        for name, shape, d in [
            ("dQT", [128, 4, S], BF16), ("dKT", [128, 2, S], BF16),
            ("dVn", [128, 16, 2, 129], BF16),
            ("detT", [128, 8, 8, 128], BF16),
            ("du", [128, 4, 128], BF16), ("dt8", [128, 4, 128], BF16),
            ("dt8T", [128, 4, 128], BF16),
        ]:
            DBG[name] = nc.dram_tensor(name, shape, d,
                                       kind="ExternalOutput").ap()

    with tile.TileContext(nc) as tc:
        _body(tc, io, out, lam)
    nc.compile()
    return nc


def _body(tc, io, out, lam):
    nc = tc.nc
    with ExitStack() as top:
        persist = top.enter_context(tc.tile_pool(name="persist", bufs=1))
        QT = persist.tile([128, 4, S], BF16)
        KT = persist.tile([128, 2, S], BF16)
        Vn = persist.tile([128, 16, 2, 129], BF16)
        # ones column for the free softmax denominators (bf16 1.0 = 0x3F80)
        nc.gpsimd.memset(Vn[:, :, :, 128:129].bitcast(mybir.dt.uint16), 0x3F80)

        _stage_a(tc, io, QT, KT, Vn)
        if DBG is not None:
            nc.sync.dma_start(DBG["dQT"], QT[:])
            nc.sync.dma_start(DBG["dKT"], KT[:])
            nc.sync.dma_start(DBG["dVn"], Vn[:])
        _stage_bc(tc, io, out, QT, KT, Vn, lam)


# ------------------------------------------------------------------- A stage

def _rope(tc, a, pq, dest_ap, ssl):
    """c0 = evac(pq); dest = c0*cs + shuffle(c0*sn)."""
    nc = tc.nc
    W = 1024
    c0 = a["tmp"].tile([128, W], BF16, tag="c0", name="c0")
    nc.scalar.activation(c0[:], pq[:], AF.Copy, bias=0.0, scale=1.0)
    t2 = a["tmp"].tile([128, W], BF16, tag="t2", name="t2")
    t2s = a["tmp"].tile([128, W], BF16, tag="t2s", name="t2s")
    nc.gpsimd.tensor_mul(t2[:], c0[:], a["sn"][:, ssl])
    nc.vector.stream_shuffle(t2s[:], t2[:], _SWAP16)
    nc.gpsimd.tensor_mul(c0[:], c0[:], a["cs"][:, ssl])
    nc.vector.tensor_add(dest_ap, c0[:], t2s[:])


def _mm3(nc, pq_ap, terms, nchunks, nsz):
    """24 DoubleRow matmuls per 256-col chunk: 3 (x,w)-terms x 8 k-pairs.

    Each psum region's accumulation group must be temporally contiguous:
    a `start` on ANY region marks its whole 2KB psum bank pending-zero,
    so an interleaved sibling group would restart mid-accumulation."""
    last_t = len(terms) - 1
    for n in range(nchunks):
        nsl = slice(n * nsz, (n + 1) * nsz)
        for ti, (xt, wt, wsl) in enumerate(terms):
            for t in range(8):
                nc.tensor.matmul(
                    pq_ap[:, nsl],
                    lhsT=wt[:, 2 * t:2 * t + 2, wsl],
                    rhs=xt[:, 2 * t:2 * t + 2, nsl],
                    start=(ti == 0 and t == 0),
                    stop=(ti == last_t and t == 7),
                    perf_mode=DR)


def _stage_a(tc, io, QT, KT, Vn):
    nc = tc.nc
    with ExitStack() as actx:
        a = {}
        a["xp"] = actx.enter_context(tc.tile_pool(name="xp", bufs=2))
        a["trig"] = actx.enter_context(tc.tile_pool(name="trig", bufs=1))
        a["tmp"] = actx.enter_context(tc.tile_pool(name="ropetmp", bufs=1))
        a["wp"] = actx.enter_context(tc.tile_pool(name="wqk", bufs=2))
        a["wvp"] = actx.enter_context(tc.tile_pool(name="wvp", bufs=1))
        with ExitStack() as pctx:
            psqk = pctx.enter_context(tc.tile_pool(name="ps_qk", bufs=2,
                                                   space="PSUM"))
            psv = pctx.enter_context(tc.tile_pool(name="ps_v", bufs=2,
                                                  space="PSUM"))
            x3 = {n: io[n].rearrange("(a p) s -> p a s", p=128)
                  for n in ("xhi", "xlo", "xh2")}
            wq3 = {n: io["wq_" + n].rearrange("(a p) c -> p a c", p=128)
                   for n in ("hi", "lo")}
            wk3 = {n: io["wk_" + n].rearrange("(a p) c -> p a c", p=128)
                   for n in ("hi", "lo")}
            wv3 = {n: io["wv_" + n].rearrange("(a p) c -> p a c", p=128)
                   for n in ("hi", "lo")}

            # x half 0, chunked so the first matmuls start early
            xh0 = {}
            for n in ("xhi", "xlo", "xh2"):
                xh0[n] = a["xp"].tile([128, 16, 1024], FP8, tag=n,
                                      name=f"{n}0")
            nc.sync.dma_start(xh0["xhi"][:, 0:4, :], x3["xhi"][:, 0:4, 0:1024])
            wk_t = {}
            for n in ("hi", "lo"):
                wk_t[n] = a["wp"].tile([128, 16, 128], FP8, tag="wk" + n,
                                       name=f"wk{n}00")
                nc.sync.dma_start(wk_t[n][:], wk3[n][:, :, 0:128])
            nc.sync.dma_start(xh0["xhi"][:, 4:16, :], x3["xhi"][:, 4:16, 0:1024])
            nc.sync.dma_start(xh0["xlo"][:], x3["xlo"][:, :, 0:1024])
            cs = a["trig"].tile([128, S], BF16)
            sn = a["trig"].tile([128, S], BF16)
            nc.sync.dma_start(cs[:, 0:1024], io["cs128"][:, 0:1024])
            nc.sync.dma_start(sn[:, 0:1024], io["sn128"][:, 0:1024])
            nc.sync.dma_start(xh0["xh2"][:], x3["xh2"][:, :, 0:1024])
            wv_t = {}
            for n in ("hi", "lo"):
                wv_t[n] = a["wvp"].tile([128, 16, 256], FP8, name=f"wv{n}")
                nc.sync.dma_start(wv_t[n][:], wv3[n][:])
            nc.sync.dma_start(cs[:, 1024:2048], io["cs128"][:, 1024:2048])
            nc.sync.dma_start(sn[:, 1024:2048], io["sn128"][:, 1024:2048])
            a["cs"], a["sn"] = cs, sn

            for sq in (0, 1):
                ssl = slice(sq * 1024, sq * 1024 + 1024)
                if sq == 0:
                    xh = xh0
                else:
                    xh = {}
                    for n in ("xhi", "xlo", "xh2"):
                        xh[n] = a["xp"].tile([128, 16, 1024], FP8, tag=n,
                                             name=f"{n}1")
                        nc.sync.dma_start(xh[n][:], x3[n][:, :, ssl])
                # K: 2 kv tiles
                for kt_i in range(2):
                    if sq == 0 and kt_i == 0:
                        wct = wk_t
                    else:
                        wct = {}
                        for n in ("hi", "lo"):
                            wct[n] = a["wp"].tile([128, 16, 128], FP8,
                                                  tag="wk" + n,
                                                  name=f"wk{n}{sq}{kt_i}")
                            nc.sync.dma_start(
                                wct[n][:],
                                wk3[n][:, :, kt_i * 128:(kt_i + 1) * 128])
                    pq = psqk.tile([128, 1024], F32, tag="psqk", name="pqk")
                    terms = [(xh["xhi"], wct["hi"], slice(None)),
                             (xh["xlo"], wct["hi"], slice(None)),
                             (xh["xh2"], wct["lo"], slice(None))]
                    _mm3(nc, pq, terms, 4, 256)
                    _rope(tc, a, pq, KT[:, kt_i, ssl], ssl)
                # V: 8 s-tiles (lhsT = x chunk, rhs = wv)
                for st in range(8):
                    pv = psv.tile([128, 256], F32, tag="psv", name="psv")
                    for ti, (xn, wn) in enumerate(
                            [("xhi", "hi"), ("xlo", "hi"), ("xh2", "lo")]):
                        for t in range(8):
                            nc.tensor.matmul(
                                pv[:],
                                lhsT=xh[xn][:, 2 * t:2 * t + 2,
                                            st * 128:(st + 1) * 128],
                                rhs=wv_t[wn][:, 2 * t:2 * t + 2, :],
                                start=(ti == 0 and t == 0),
                                stop=(ti == 2 and t == 7),
                                perf_mode=DR)
                    for kv in range(2):
                        nc.vector.tensor_copy(
                            Vn[:, sq * 8 + st, kv, 0:128],
                            pv[:, kv * 128:(kv + 1) * 128])
                # Q: 4 head tiles
                for ct in range(4):
                    wct = {}
                    for n in ("hi", "lo"):
                        wct[n] = a["wp"].tile([128, 16, 128], FP8,
                                              tag="wq" + n,
                                              name=f"wq{n}{sq}{ct}")
                        nc.sync.dma_start(
                            wct[n][:],
                            wq3[n][:, :, ct * 128:(ct + 1) * 128])
                    pq = psqk.tile([128, 1024], F32, tag="psqk", name="pqq")
                    terms = [(xh["xhi"], wct["hi"], slice(None)),
                             (xh["xlo"], wct["hi"], slice(None)),
                             (xh["xh2"], wct["lo"], slice(None))]
                    _mm3(nc, pq, terms, 4, 256)
                    _rope(tc, a, pq, QT[:, ct, ssl], ssl)


# ------------------------------------------------------------------ BC stage

def _stage_bc(tc, io, out, QT, KT, Vn, lam):
    """Attention + norm + wo, pipelined over 16 query tiles, skew 1/2.

    step s: scoresT+exp(qt=s) | pv+norm(qt=s-1) | at-split+wo+out(qt=s-2).
    """
    nc = tc.nc
    H_ORDER = (0, 2, 1, 3)
    with ExitStack() as ctx:
        etp = [ctx.enter_context(tc.tile_pool(name="etpA", bufs=1)),
               ctx.enter_context(tc.tile_pool(name="etpB", bufs=1))]
        wotp = ctx.enter_context(tc.tile_pool(name="wotp", bufs=1))
        colp = ctx.enter_context(tc.tile_pool(name="colp", bufs=3))
        up = ctx.enter_context(tc.tile_pool(name="up", bufs=2))
        t8p = ctx.enter_context(tc.tile_pool(name="t8p", bufs=2))
        t8Tp = ctx.enter_context(tc.tile_pool(name="t8Tp", bufs=2))
        atxp = ctx.enter_context(tc.tile_pool(name="atxp", bufs=2))
        outwp = ctx.enter_context(tc.tile_pool(name="outwp", bufs=2))
        magp = ctx.enter_context(tc.tile_pool(name="magic", bufs=1))
        magicT = magp.tile([128, 4], mybir.dt.uint32, name="magicT")
        nc.gpsimd.memset(magicT[:], 0x5F3759DF)

        wot = {}
        for n in ("hi", "lo"):
            wot[n] = wotp.tile([128, 4, S], FP8, name=f"wot{n}")
            nc.sync.dma_start(wot[n][:], io["wo_" + n].rearrange(
                "(a p) c -> p a c", p=128))

        pss = ctx.enter_context(tc.tile_pool(name="pss", bufs=2,
                                             space="PSUM"))
        ppvp = ctx.enter_context(tc.tile_pool(name="ppv", bufs=1,
                                              space="PSUM"))
        psop = ctx.enter_context(tc.tile_pool(name="pso", bufs=1,
                                              space="PSUM"))

        etT_t = [None] * NQT
        t8T_t = [None] * NQT

        for step in range(NQT + 2):
            fr = step if step < NQT else None
            mid = step - 1 if 1 <= step <= NQT else None
            bk = step - 2 if step >= 2 else None

            if fr is not None:
                etT = etp[fr % 2].tile([128, 8, 8, 128], BF16, tag="et",
                                       name=f"et{fr}")
                etT_t[fr] = etT
            if mid is not None:
                ppvA = ppvp.tile([128, 512], F32, tag="ppvA")
                ppvB = ppvp.tile([128, 512], F32, tag="ppvB")
                ppvC = ppvp.tile([128, 512], F32, tag="ppvC")

                def slot(h, j):
                    flat = h * 2 + j
                    t = (ppvA, ppvB, ppvC)[flat // 3]
                    off = (flat % 3) * 129
                    return t[:, off:off + 129]
                u = up.tile([128, 4, 128], BF16, tag="u")
                v2 = up.tile([128, 128], BF16, tag="v2")
                usq = up.tile([128, 128], BF16, tag="usq")
                dc = colp.tile([128, 4], F32, tag="dc")
                rec = colp.tile([128, 4], F32, tag="rec")
                rrl = colp.tile([128, 4], F32, tag="rrl")
                msum = colp.tile([128, 4], F32, tag="msum")
                tcol = colp.tile([128, 4], F32, tag="tcol")
                ya = colp.tile([128, 4], F32, tag="ya")
                aa = colp.tile([128, 4], F32, tag="aa")
                shu = colp.tile([128, 4], mybir.dt.uint32, tag="shu")
            if bk is not None:
                t8Tb = t8T_t[bk]
                athi = atxp.tile([128, 4, 128], FP8, tag="athi")
                atlo = atxp.tile([128, 4, 128], FP8, tag="atlo")
                ath4 = atxp.tile([128, 4, 128], FP8, tag="ath4")
                nc.vector.tensor_copy(athi[:], t8Tb[:])
                nc.vector.tensor_sub(atlo[:], t8Tb[:], athi[:])
                nc.vector.tensor_scalar(ath4[:], athi[:], 0.03125, None,
                                        op0=ALU.mult)
                outw = outwp.tile([128, 2048], BF16, tag="outw")

            for beat in range(8):
                # frontend: transposed scores + exp for one (h, j)
                if fr is not None:
                    hj = beat
                    j, h = divmod(hj, 4)
                    kvl, rho = h // 2, h % 2
                    jsl = slice(j * 64, j * 64 + 64)
                    qsl = slice(fr * 128, fr * 128 + 128)
                    psc = pss.tile([128, 8, 128], F32, tag="sc")
                    for kt in range(8):
                        kof = rho * 1024 + kt * 128
                        nc.tensor.matmul(
                            psc[:, kt, :],
                            lhsT=KT[jsl, kvl, kof:kof + 128],
                            rhs=QT[jsl, h, qsl],
                            start=True, stop=True)
                    nc.scalar.activation(etT[:, hj, :, :], psc[:], AF.Exp,
                                         bias=0.0, scale=float(ESC))
                # middle: pv for one (h, j); after j=1 the u'/norm column ops
                if mid is not None:
                    h = H_ORDER[beat // 2]
                    j = beat % 2
                    hj = j * 4 + h
                    kvl, rho = h // 2, h % 2
                    etm = etT_t[mid]
                    pdst = slot(h, j)
                    for kt in range(8):
                        nc.tensor.matmul(
                            pdst,
                            lhsT=etm[:, hj, kt, :],
                            rhs=Vn[:, rho * 8 + kt, kvl, :],
                            start=(kt == 0), stop=(kt == 7))
                    if j == 1:
                        s0, s1 = slot(h, 0), slot(h, 1)
                        nc.vector.reciprocal(rec[:, h:h + 1], s1[:, 128:129])
                        nc.vector.scalar_tensor_tensor(
                            rrl[:, h:h + 1], s0[:, 128:129], float(lam),
                            rec[:, h:h + 1], op0=ALU.mult, op1=ALU.mult)
                        nc.vector.tensor_scalar_mul(
                            v2[:], s1[:, 0:128], rrl[:, h:h + 1])
                        nc.vector.tensor_sub(u[:, h, :], v2[:], s0[:, 0:128])
                        nc.vector.scalar_tensor_tensor(
                            usq[:], u[:, h, :], 1.0, u[:, h, :],
                            op0=ALU.mult, op1=ALU.mult,
                            accum_out=msum[:, h:h + 1])
                        nc.vector.tensor_copy(dc[:, h:h + 1], s0[:, 128:129])
                # backend: one 256-col chunk of the fp8 wo matmul
                if bk is not None:
                    nch = beat
                    if nch % 2 == 0:
                        pso = psop.tile([128, 512], F32, tag="pso")
                    reg = pso[:, (nch % 2) * 256:(nch % 2) * 256 + 256]
                    csl = slice(nch * 256, (nch + 1) * 256)
                    i = 0
                    for ax, wn in ((athi, "hi"), (atlo, "hi"), (ath4, "lo")):
                        for r in range(2):
                            nc.tensor.matmul(
                                reg,
                                lhsT=ax[:, 2 * r:2 * r + 2, :],
                                rhs=wot[wn][:, 2 * r:2 * r + 2, csl],
                                start=(i == 0), stop=(i == 5),
                                perf_mode=DR)
                            i += 1
                    if nch % 2 == 1:
                        nc.vector.tensor_copy(
                            outw[:, (nch - 1) * 256:(nch + 1) * 256], pso[:])

            if mid is not None:
                # ya = rsqrt(msum/8192 + 64*eps*d1^2) = rsqrt(true)/8
                # Quake bit-trick + two Newton steps on the DVE
                nc.vector.scalar_tensor_tensor(
                    tcol[:], dc[:], float(64.0 * EPS), dc[:],
                    op0=ALU.mult, op1=ALU.mult)
                nc.vector.scalar_tensor_tensor(
                    tcol[:], msum[:], float(1.0 / 8192.0), tcol[:],
                    op0=ALU.mult, op1=ALU.add)
                nc.vector.tensor_scalar(shu[:], tcol[:].bitcast(
                    mybir.dt.uint32), 1, None, op0=ALU.arith_shift_right)
                nc.vector.tensor_sub(ya[:].bitcast(mybir.dt.uint32),
                                     magicT[:], shu[:])
                for _ in range(2):
                    nc.vector.tensor_mul(aa[:], ya[:], ya[:])
                    nc.vector.tensor_mul(aa[:], aa[:], tcol[:])
                    nc.vector.tensor_scalar(aa[:], aa[:], -0.5, 1.5,
                                            op0=ALU.mult, op1=ALU.add)
                    nc.vector.tensor_mul(ya[:], ya[:], aa[:])
                t8 = t8p.tile([128, 4, 128], BF16, tag="t8")
                for h in range(4):
                    nc.vector.tensor_scalar_mul(t8[:, h, :], u[:, h, :],
                                                ya[:, h:h + 1])
                t8T = t8Tp.tile([128, 4, 128], BF16, tag="t8T")
                t8T_t[mid] = t8T
                nc.sync.dma_start_transpose(t8T[:], t8[:])
                if DBG is not None and mid == 0:
                    nc.sync.dma_start(DBG["detT"], etT_t[0][:])
                    nc.sync.dma_start(DBG["du"], u[:])
                    nc.sync.dma_start(DBG["dt8"], t8[:])
                    nc.sync.dma_start(DBG["dt8T"], t8T[:])

            if bk is not None:
                nc.gpsimd.dma_start(out[bk * 128:(bk + 1) * 128, :], outw[:])


# ---------------------------------------------------------------- host side

# row (q*32 + c*16 + i) within a branch <-> head-dim 2*(16q+i)+c
_PERM64 = np.empty(64, np.int64)
for _q in range(2):
    for _c in range(2):
        for _i in range(16):
            _PERM64[_q * 32 + _c * 16 + _i] = 2 * (16 * _q + _i) + _c


def _hilo_w(w, boost):
    w4 = (boost * w).astype(np.float32)
    hi = w4.astype(F8NP)
    lo = (32.0 * (w4 - hi.astype(np.float32))).astype(F8NP)
    return hi, lo


def make_core_inputs(core, x, wq, wk, wv, wo, subln_w, lambda_init,
                     freqs_cos, freqs_sin):
    b, g = divmod(core, 4)
    qcols = np.empty(512, np.int64)
    for hl in range(4):
        for j in range(2):
            qcols[hl * 128 + j * 64:hl * 128 + j * 64 + 64] = \
                ((4 * g + hl) * 2 + j) * 64 + _PERM64
    kcols = np.empty(256, np.int64)
    for kvl in range(2):
        for j in range(2):
            kcols[kvl * 128 + j * 64:kvl * 128 + j * 64 + 64] = \
                ((2 * g + kvl) * 2 + j) * 64 + _PERM64
    vcols = np.arange(256) + 2 * g * 128

    cosT = np.ascontiguousarray(freqs_cos.T.astype(np.float32))  # [32, S]
    sinT = np.ascontiguousarray(freqs_sin.T.astype(np.float32))
    cs64 = np.concatenate([cosT[0:16], cosT[0:16],
                           cosT[16:32], cosT[16:32]], axis=0)
    sn64 = np.concatenate([sinT[0:16], -sinT[0:16],
                           sinT[16:32], -sinT[16:32]], axis=0)

    xT = np.ascontiguousarray(x[b].T.astype(np.float32))
    x16 = 16.0 * xT
    xhi = x16.astype(F8NP)
    xlo = (x16 - xhi.astype(np.float32)).astype(F8NP)
    xh2 = (0.5 * xT).astype(F8NP)

    wq_hi, wq_lo = _hilo_w(wq[:, qcols].astype(np.float32), 4.0)
    wk_hi, wk_lo = _hilo_w(wk[:, kcols].astype(np.float32), 4.0)
    wv_hi, wv_lo = _hilo_w(wv[:, vcols].astype(np.float32), 4.0)

    # wo rows carry subln*(1-lambda_init) and the global sign flip (u' = -u)
    wo_eff = wo[512 * g: 512 * g + 512, :].astype(np.float32).copy()
    wo_eff *= -np.tile(subln_w.astype(np.float32)
                       * (1.0 - np.float32(np.asarray(lambda_init)[0])),
                       4)[:, None]
    wo_hi, wo_lo = _hilo_w(wo_eff, 16.0)

    return {
        "xhi": xhi, "xlo": xlo, "xh2": xh2,
        "wq_hi": wq_hi, "wq_lo": wq_lo,
        "wk_hi": wk_hi, "wk_lo": wk_lo,
        "wv_hi": wv_hi, "wv_lo": wv_lo,
        "wo_hi": wo_hi, "wo_lo": wo_lo,
        "cs128": np.tile(cs64, (2, 1)).astype(BFNP),
        "sn128": np.tile(sn64, (2, 1)).astype(BFNP),
    }


def compute_lambda(lambda_q1, lambda_k1, lambda_q2, lambda_k2, lambda_init):
    l1 = np.exp(np.sum(np.float32(lambda_q1) * np.float32(lambda_k1),
                       dtype=np.float32))
    l2 = np.exp(np.sum(np.float32(lambda_q2) * np.float32(lambda_k2),
                       dtype=np.float32))
    return float(l1 - l2 + np.float32(np.asarray(lambda_init)[0]))


def kernel(x, wq, wk, wv, wo, lambda_q1, lambda_k1, lambda_q2, lambda_k2,
           lambda_init, subln_w, freqs_cos, freqs_sin):
    global LAST_RESULTS
    x = np.asarray(x); wq = np.asarray(wq); wk = np.asarray(wk)
    wv = np.asarray(wv); wo = np.asarray(wo)
    lam = compute_lambda(lambda_q1, lambda_k1, lambda_q2, lambda_k2,
                         lambda_init)

    nc = build_program(lam)
    in_maps = [make_core_inputs(c, x, wq, wk, wv, wo,
                                np.asarray(subln_w), np.asarray(lambda_init),
                                np.asarray(freqs_cos), np.asarray(freqs_sin))
               for c in range(NCORES)]
    res = run_bass_kernel_spmd(nc, in_maps, list(range(NCORES)), trace=TRACE)
    LAST_RESULTS = res
    outs = [res.results[c]["out"] for c in range(NCORES)]
    full = np.empty((B, S, DIM), np.float32)
    for b in range(B):
        full[b] = (outs[4 * b].astype(np.float32)
                   + outs[4 * b + 1].astype(np.float32)
                   + outs[4 * b + 2].astype(np.float32)
                   + outs[4 * b + 3].astype(np.float32)) * (1.0 / 128.0)
    return full
